# revision 14
# baseline (speedup 1.0000x reference)
"""Trainium2 Bass kernel for nn_BigramModel (6-layer GPT-style transformer).

Strategy: data-parallel over 8 NeuronCores (B=128 -> 16 batch rows/core).
On-device layout is feature-major ("transposed"): activations live as
[C, tokens] so every linear layer is a plain PE matmul chain with no
transposes anywhere.  All matmuls run in fp16 (fp32 PSUM accumulation);
the residual stream, layernorm statistics and softmax denominators stay
fp32.

 - LayerNorm: per-token stats via ones-column PE matmuls (sum(x), sum(x^2)),
   rsqrt computed as exp(-0.5*ln(q)) so ACT only ever needs the
   natural_log_exp table set; normalization applied as x*Abc + Bbc where
   Abc/Bbc are rank-1 (gamma x A-row) built by K=1 PE matmuls.
 - Attention: scores computed transposed (scoresT[k, q]) per (batch, head),
   causal masking on the fp16 exp tiles, denominators broadcast with
   ones-block matmuls, 1/denom folded into the AV output eviction.
"""

import os
import numpy as np
from contextlib import ExitStack

import concourse.bass as bass
import concourse.tile as tile
from concourse import bacc, mybir
from concourse.bass_utils import run_bass_kernel_spmd

AF = mybir.ActivationFunctionType
ALU = mybir.AluOpType
f16 = mybir.dt.float16
f32 = mybir.dt.float32

B, T, V, C, H, HD, L = 128, 256, 100, 384, 6, 64, 6
FF = 4 * C
NCORE = 8
BL = B // NCORE          # 16 batch rows per core
NT = BL * T              # 4096 tokens per core
CH = 512                 # tokens per pipeline chunk (2 batch rows)
NCH = NT // CH           # 8 chunks
CC = C // 128            # 3 feature chunks
FC = FF // 128           # 12 ffn feature chunks
LNEPS_C = (384.0 ** 2) * 1e-5   # eps term for 384^2 * var
LOG384 = float(np.log(384.0))


def _patch_act_tables():
    """Force every ACT instruction onto the natural_log_exp_and_others table
    set (covers exp/ln/copy/identity) so the kernel loads exactly one ACT
    table instead of thrashing between exp- and ln-sets at every layernorm."""
    import concourse.bacc as _bacc
    if getattr(_bacc, "_act_tables_patched", False):
        return
    orig = _bacc.get_activation_tables

    def patched(arch):
        tabs = orig(arch)
        keep = "natural_log_exp_and_others"
        if keep in tabs:
            tabs = {k: (v if k == keep else set()) for k, v in tabs.items()}
        return tabs

    _bacc.get_activation_tables = patched
    _bacc._act_tables_patched = True


def build(n_layers=L, n_chunks=NCH, taps=()):
    _patch_act_tables()
    nc = bacc.Bacc("TRN2", target_bir_lowering=False, debug=False)

    x0_d = nc.dram_tensor("x0T", [C, NT], f32, kind="ExternalInput")
    wq_d, wk_d, wv_d, wo_d, w1_d, w2_d = [], [], [], [], [], []
    bias4_d, vbr_d, b1e_d = [], [], []
    for l in range(n_layers):
        wq_d.append(nc.dram_tensor(f"wq{l}", [C, C], f16, kind="ExternalInput"))
        wk_d.append(nc.dram_tensor(f"wk{l}", [C, C], f16, kind="ExternalInput"))
        wv_d.append(nc.dram_tensor(f"wv{l}", [C, C], f16, kind="ExternalInput"))
        wo_d.append(nc.dram_tensor(f"wo{l}", [C, C], f16, kind="ExternalInput"))
        w1_d.append(nc.dram_tensor(f"w1{l}", [C, FF], f16, kind="ExternalInput"))
        w2_d.append(nc.dram_tensor(f"w2{l}", [FF, C], f16, kind="ExternalInput"))
        bias4_d.append(nc.dram_tensor(f"bias4_{l}", [C, 6], f32, kind="ExternalInput"))
        vbr_d.append(nc.dram_tensor(f"vbr{l}", [1, C], f16, kind="ExternalInput"))
        b1e_d.append(nc.dram_tensor(f"b1e{l}", [128, FC], f32, kind="ExternalInput"))
    mska_d = nc.dram_tensor("mska", [128, 768], f16, kind="ExternalInput")
    gfc_d = nc.dram_tensor("gfc", [C, 1], f32, kind="ExternalInput")
    wlm_d = nc.dram_tensor("wlm", [C, V], f16, kind="ExternalInput")
    lmb_d = nc.dram_tensor("lmb", [V, 1], f32, kind="ExternalInput")
    out_d = nc.dram_tensor("logitsT", [V, NT], f32, kind="ExternalOutput")
    tap_d = {}
    for tname in taps:
        tap_d[tname] = nc.dram_tensor(tname, [C, NT], f32, kind="ExternalOutput")

    with tile.TileContext(nc) as tc, ExitStack() as ctx:
        const = ctx.enter_context(tc.tile_pool(name="const", bufs=1))
        wpool = ctx.enter_context(tc.tile_pool(name="w", bufs=2))
        xpool = ctx.enter_context(tc.tile_pool(name="x", bufs=1))
        sc = ctx.enter_context(tc.tile_pool(name="sc", bufs=2))
        lnp = ctx.enter_context(tc.tile_pool(name="lnp", bufs=3))
        xsp = ctx.enter_context(tc.tile_pool(name="xsp", bufs=3))
        vpool = ctx.enter_context(tc.tile_pool(name="vp", bufs=5))
        mpool = ctx.enter_context(tc.tile_pool(name="m1p", bufs=13))
        epool = ctx.enter_context(tc.tile_pool(name="ep", bufs=5))
        psA = ctx.enter_context(tc.tile_pool(name="psA", bufs=2, space="PSUM"))
        psQ = ctx.enter_context(tc.tile_pool(name="psQ", bufs=1, space="PSUM"))
        psB = ctx.enter_context(tc.tile_pool(name="psB", bufs=2, space="PSUM"))
        psS = ctx.enter_context(tc.tile_pool(name="psS", bufs=3, space="PSUM"))

        # ---- constants
        mska = const.tile([128, 768], f16)           # causal masks for e-mega
        nc.sync.dma_start(mska[:], mska_d.ap())
        onesL = const.tile([128, 128], f16)          # cols 0-63 ones
        nc.vector.memset(onesL[:], 0.0)
        nc.vector.memset(onesL[:, 0:64], 1.0)
        onesR = const.tile([128, 128], f16)          # cols 64-127 ones
        nc.vector.memset(onesR[:], 0.0)
        nc.vector.memset(onesR[:, 64:128], 1.0)
        ones_col = const.tile([128, 1], f16)
        nc.vector.memset(ones_col[:], 1.0)
        ones_row = const.tile([1, 128], f16)
        nc.vector.memset(ones_row[:], 1.0)
        lneps_t = const.tile([128, 1], f32)
        nc.vector.memset(lneps_t[:], LNEPS_C)
        log384_t = const.tile([128, 1], f32)
        nc.vector.memset(log384_t[:], LOG384)
        gfc = []
        for i in range(CC):
            g = const.tile([128, 1], f32, tag=f"gfc{i}")
            nc.sync.dma_start(g[:], gfc_d.ap()[128 * i:128 * (i + 1), :])
            gfc.append(g)
        wlm_t = []
        for i in range(CC):
            w = const.tile([128, V], f16, tag=f"wlm{i}")
            nc.sync.dma_start(w[:], wlm_d.ap()[128 * i:128 * (i + 1), :])
            wlm_t.append(w)
        lmb_t = const.tile([128, 1], f32)
        nc.sync.dma_start(lmb_t[0:V, :], lmb_d.ap())

        # ---- resident residual stream (feature-major, fp32)
        x_t = []
        for i in range(CC):
            xt = xpool.tile([128, NT], f32, tag=f"x{i}")
            nc.sync.dma_start(xt[:], x0_d.ap()[128 * i:128 * (i + 1), :])
            x_t.append(xt)

        def layer_norm_chunk(t0, g_aps):
            """LN of x[:, t0:t0+CH]; returns 3 fp16 [128, CH] tiles (gamma applied,
            beta NOT applied - folded into consumers). g_aps: per-chunk [128,1] f32."""
            stats = psS.tile([128, CH], f32, tag="st")
            for i in range(CC):
                xs = x_t[i][:, t0:t0 + CH]
                x16 = xsp.tile([128, CH], f16, tag=f"x16_{i}")
                nc.gpsimd.tensor_copy(x16[:], xs)
                xsq = sc.tile([128, CH], f16, tag=f"xsq_{i}")
                nc.vector.tensor_mul(xsq[:], xs, xs)
                nc.tensor.matmul(stats[0:1, :], ones_col[:], x16[:],
                                 start=(i == 0), stop=(i == CC - 1))
                nc.tensor.matmul(stats[32:33, :], ones_col[:], xsq[:],
                                 start=(i == 0), stop=(i == CC - 1))
            sq = lnp.tile([1, CH], f32, tag="sq")
            nc.scalar.activation(sq[:], stats[0:1, :], AF.Square)
            qr = lnp.tile([1, CH], f32, tag="qr")
            nc.vector.scalar_tensor_tensor(qr[:], stats[32:33, :], 384.0, sq[:],
                                           op0=ALU.mult, op1=ALU.subtract)
            lg = lnp.tile([1, CH], f32, tag="lgr")
            nc.scalar.activation(lg[:], qr[:], AF.Ln, bias=lneps_t[0:1, :], scale=1.0)
            ar = lnp.tile([1, CH], f16, tag="ar")
            nc.scalar.activation(ar[:], lg[:], AF.Exp, bias=log384_t[0:1, :], scale=-0.5)
            br = lnp.tile([1, CH], f16, tag="br")
            nc.vector.scalar_tensor_tensor(br[:], stats[0:1, :], -1.0 / 384.0, ar[:],
                                           op0=ALU.mult, op1=ALU.mult)
            ab = psB.tile([128, CH], f32, tag="b")
            nc.tensor.matmul(ab[:], ones_row[:], ar[:], start=True, stop=True)
            bb = psB.tile([128, CH], f32, tag="b")
            nc.tensor.matmul(bb[:], ones_row[:], br[:], start=True, stop=True)
            xh = []
            for i in range(CC):
                tt = sc.tile([128, CH], f32, tag="t1")
                nc.vector.scalar_tensor_tensor(tt[:], x_t[i][:, t0:t0 + CH],
                                               g_aps[i], ab[:],
                                               op0=ALU.mult, op1=ALU.mult)
                xh_i = sc.tile([128, CH], f16, tag=f"xh{i}")
                nc.vector.scalar_tensor_tensor(xh_i[:], bb[:], g_aps[i], tt[:],
                                               op0=ALU.mult, op1=ALU.add)
                xh.append(xh_i)
            return xh

        for l in range(n_layers):
            # ---- load layer weights (double-buffered via tags)
            def wload(dram, n_t, width, name):
                ts = []
                for i in range(n_t):
                    w = wpool.tile([128, width], f16, tag=f"{name}{i}")
                    nc.sync.dma_start(w[:], dram.ap()[128 * i:128 * (i + 1), :])
                    ts.append(w)
                return ts
            wq_t = wload(wq_d[l], CC, C, "wq")
            wk_t = wload(wk_d[l], CC, C, "wk")
            wv_t = wload(wv_d[l], CC, C, "wv")
            wo_t = wload(wo_d[l], CC, C, "wo")
            w1_t = wload(w1_d[l], CC, FF, "w1")
            w2_t = wload(w2_d[l], FC, C, "w2")
            bias4_t = []
            for i in range(CC):
                bt = wpool.tile([128, 6], f32, tag=f"b4_{i}")
                nc.sync.dma_start(bt[:], bias4_d[l].ap()[128 * i:128 * (i + 1), :])
                bias4_t.append(bt)
            vbr = wpool.tile([1, C], f16, tag="vbr")
            nc.sync.dma_start(vbr[:], vbr_d[l].ap())
            b1e = wpool.tile([128, FC], f32, tag="b1e")
            nc.sync.dma_start(b1e[:], b1e_d[l].ap())

            for ch in range(n_chunks):
                t0 = ch * CH
                # ======== LN1 + QKV ========
                xh = layer_norm_chunk(t0, [bias4_t[i][:, 4:5] for i in range(CC)])
                qt, kt = [], []
                for j in range(CC):           # output head-pair tiles
                    qp = psQ.tile([128, CH], f32, tag="q")
                    for i in range(CC):
                        nc.tensor.matmul(qp[:], wq_t[i][:, 128 * j:128 * (j + 1)],
                                         xh[i][:], start=(i == 0), stop=(i == CC - 1))
                    q16 = sc.tile([128, CH], f16, tag=f"qt{j}")
                    nc.scalar.activation(q16[:], qp[:], AF.Identity,
                                         bias=bias4_t[j][:, 0:1], scale=1.0)
                    qt.append(q16)
                    kp = psQ.tile([128, CH], f32, tag="q")
                    for i in range(CC):
                        nc.tensor.matmul(kp[:], wk_t[i][:, 128 * j:128 * (j + 1)],
                                         xh[i][:], start=(i == 0), stop=(i == CC - 1))
                    k16 = sc.tile([128, CH], f16, tag=f"kt{j}")
                    nc.scalar.activation(k16[:], kp[:], AF.Identity,
                                         bias=bias4_t[j][:, 1:2], scale=1.0)
                    kt.append(k16)
                vt = []
                for tb in range(CH // 128):   # token-major v tiles [128 tok, C]
                    vp = psB.tile([128, C], f32, tag="b")
                    for i in range(CC):
                        nc.tensor.matmul(vp[:], xh[i][:, 128 * tb:128 * (tb + 1)],
                                         wv_t[i][:], start=(i == 0), stop=False)
                    nc.tensor.matmul(vp[:], ones_row[:], vbr[:], start=False, stop=True)
                    v16 = vpool.tile([128, C], f16, tag="v16")
                    nc.scalar.copy(v16[:], vp[:])
                    vt.append(v16)

                # ======== attention ========
                oT = []
                for j in range(CC):
                    ot = sc.tile([128, CH], f16, tag=f"oT{j}")
                    oT.append(ot)
                for hb in range(CH // T):     # batch row within chunk
                    q0 = hb * T
                    for hp in range(CC):      # head pair
                        em = epool.tile([128, 768], f16, tag="e")
                        for hh in range(2):
                            rb = 64 * hh
                            sp = psA.tile([128, 2 * T], f32, tag="a")
                            nc.tensor.matmul(
                                sp[:, 0:T],
                                kt[hp][rb:rb + 64, q0:q0 + 128],
                                qt[hp][rb:rb + 64, q0:q0 + T],
                                start=True, stop=True)
                            nc.tensor.matmul(
                                sp[:, T:T + 128],
                                kt[hp][rb:rb + 64, q0 + 128:q0 + T],
                                qt[hp][rb:rb + 64, q0 + 128:q0 + T],
                                start=True, stop=True)
                            nc.scalar.activation(em[:, 384 * hh:384 * (hh + 1)],
                                                 sp[:, 0:384], AF.Exp, scale=0.125)
                        nc.vector.tensor_mul(em[:], em[:], mska[:])
                        ee = lambda hh, ktb: (
                            em[:, 384 * hh:384 * hh + T] if ktb == 0
                            else em[:, 384 * hh + T:384 * hh + T + 128])
                        db = psA.tile([128, 2 * T], f32, tag="a")
                        nc.tensor.matmul(db[:, 0:T], onesL[:], ee(0, 0), start=True, stop=False)
                        nc.tensor.matmul(db[:, 0:T], onesR[:], ee(1, 0), start=False, stop=False)
                        nc.tensor.matmul(db[:, 128:T], onesL[:], ee(0, 1), start=False, stop=False)
                        nc.tensor.matmul(db[:, 128:T], onesR[:], ee(1, 1), start=False, stop=True)
                        rd = sc.tile([128, T], f32, tag="rd")
                        nc.vector.reciprocal_approx_fast(rd[:], db[:, 0:T])
                        av = psA.tile([128, 2 * T], f32, tag="a")
                        for hh in range(2):
                            rb = 64 * hh
                            nc.tensor.matmul(
                                av[rb:rb + 64, 0:T],
                                vt[hb * 2][:, 128 * hp + rb:128 * hp + rb + 64],
                                ee(hh, 0), start=True, stop=False)
                            nc.tensor.matmul(
                                av[rb:rb + 64, 128:T],
                                vt[hb * 2 + 1][:, 128 * hp + rb:128 * hp + rb + 64],
                                ee(hh, 1), start=False, stop=True)
                        nc.vector.tensor_mul(oT[hp][:, q0:q0 + T], av[:, 0:T], rd[:])

                # ======== Wo + residual ========
                for co in range(CC):
                    dl = psB.tile([128, CH], f32, tag="b")
                    for ci in range(CC):
                        nc.tensor.matmul(dl[:], wo_t[ci][:, 128 * co:128 * (co + 1)],
                                         oT[ci][:], start=(ci == 0), stop=(ci == CC - 1))
                    nc.vector.scalar_tensor_tensor(
                        x_t[co][:, t0:t0 + CH], dl[:], bias4_t[co][:, 2:3],
                        x_t[co][:, t0:t0 + CH], op0=ALU.add, op1=ALU.add)

                # ======== LN2 + FFN ========
                xh2 = layer_norm_chunk(t0, [bias4_t[i][:, 5:6] for i in range(CC)])
                m1 = []
                for fo in range(FC):
                    mp = psB.tile([128, CH], f32, tag="b")
                    for ci in range(CC):
                        nc.tensor.matmul(mp[:], w1_t[ci][:, 128 * fo:128 * (fo + 1)],
                                         xh2[ci][:], start=(ci == 0), stop=(ci == CC - 1))
                    m16 = mpool.tile([128, CH], f16, tag="m1")
                    nc.scalar.activation(m16[:], mp[:], AF.Relu,
                                         bias=b1e[:, fo:fo + 1], scale=1.0)
                    m1.append(m16)
                for co in range(CC):
                    dl = psB.tile([128, CH], f32, tag="b")
                    for fo in range(FC):
                        nc.tensor.matmul(dl[:], w2_t[fo][:, 128 * co:128 * (co + 1)],
                                         m1[fo][:], start=(fo == 0), stop=(fo == FC - 1))
                    nc.vector.scalar_tensor_tensor(
                        x_t[co][:, t0:t0 + CH], dl[:], bias4_t[co][:, 3:4],
                        x_t[co][:, t0:t0 + CH], op0=ALU.add, op1=ALU.add)

        # ======== final LN + LM head ========
        for ch in range(n_chunks):
            t0 = ch * CH
            xhf = layer_norm_chunk(t0, [g[:, 0:1] for g in gfc])
            lp = psB.tile([128, CH], f32, tag="b")
            for ci in range(CC):
                nc.tensor.matmul(lp[0:V, :], wlm_t[ci][:], xhf[ci][:],
                                 start=(ci == 0), stop=(ci == CC - 1))
            lo = sc.tile([128, CH], f32, tag="lo")
            nc.scalar.activation(lo[0:V, :], lp[0:V, :], AF.Identity,
                                 bias=lmb_t[0:V, :], scale=1.0)
            nc.sync.dma_start(out_d.ap()[:, t0:t0 + CH], lo[0:V, :])

        for tname in taps:
            for i in range(CC):
                nc.sync.dma_start(tap_d[tname].ap()[128 * i:128 * (i + 1), :], x_t[i][:])

    nc.compile()
    return nc


def host_prep(inputs, n_layers=L):
    """Build the per-core input maps from the full model inputs."""
    idx = np.asarray(inputs["idx"])
    embed = np.asarray(inputs["embed"], dtype=np.float32)
    pos = np.asarray(inputs["pos_embed"], dtype=np.float32)
    Wq = np.asarray(inputs["Wq"], dtype=np.float32)
    Wk = np.asarray(inputs["Wk"], dtype=np.float32)
    Wv = np.asarray(inputs["Wv"], dtype=np.float32)
    Wo = np.asarray(inputs["Wo"], dtype=np.float32)
    bo = np.asarray(inputs["bo"], dtype=np.float32)
    W1 = np.asarray(inputs["W1"], dtype=np.float32)
    b1 = np.asarray(inputs["b1"], dtype=np.float32)
    W2 = np.asarray(inputs["W2"], dtype=np.float32)
    b2 = np.asarray(inputs["b2"], dtype=np.float32)
    ln1_g = np.asarray(inputs["ln1_g"], dtype=np.float32)
    ln1_b = np.asarray(inputs["ln1_b"], dtype=np.float32)
    ln2_g = np.asarray(inputs["ln2_g"], dtype=np.float32)
    ln2_b = np.asarray(inputs["ln2_b"], dtype=np.float32)
    lnf_g = np.asarray(inputs["lnf_g"], dtype=np.float32)
    lnf_b = np.asarray(inputs["lnf_b"], dtype=np.float32)
    Wlm = np.asarray(inputs["Wlm"], dtype=np.float32)
    blm = np.asarray(inputs["blm"], dtype=np.float32)

    x0 = embed[idx] + pos[None, :T, :]          # [B, T, C] fp32

    shared = {}
    for l in range(n_layers):
        shared[f"wq{l}"] = Wq[l].astype(np.float16)
        shared[f"wk{l}"] = Wk[l].astype(np.float16)
        shared[f"wv{l}"] = Wv[l].astype(np.float16)
        shared[f"wo{l}"] = Wo[l].astype(np.float16)
        shared[f"w1{l}"] = W1[l].astype(np.float16)
        shared[f"w2{l}"] = W2[l].astype(np.float16)
        shared[f"bias4_{l}"] = np.stack(
            [Wq[l].T @ ln1_b[l], Wk[l].T @ ln1_b[l], bo[l], b2[l],
             ln1_g[l], ln2_g[l]],
            axis=1).astype(np.float32)
        shared[f"vbr{l}"] = (ln1_b[l] @ Wv[l])[None, :].astype(np.float16)
        b1e = b1[l] + W1[l].T @ ln2_b[l]
        shared[f"b1e{l}"] = np.ascontiguousarray(
            b1e.reshape(FC, 128).T).astype(np.float32)
    k_i = np.arange(128)[:, None]
    q_i = np.arange(128)[None, :]
    tri = (k_i <= q_i).astype(np.float16)
    ones = np.ones((128, 128), np.float16)
    zeros = np.zeros((128, 128), np.float16)
    half = np.concatenate([tri, ones, tri], axis=1)
    shared["mska"] = np.concatenate([half, half], axis=1)
    shared["gfc"] = lnf_g[:, None].astype(np.float32)
    shared["wlm"] = Wlm.astype(np.float16)
    shared["lmb"] = (blm + Wlm.T @ lnf_b)[:, None].astype(np.float32)

    in_maps = []
    for c in range(NCORE):
        x0c = x0[c * BL:(c + 1) * BL].reshape(NT, C).T
        m = dict(shared)
        m["x0T"] = np.ascontiguousarray(x0c, dtype=np.float32)
        in_maps.append(m)
    return in_maps


_CACHE = {}
LAST_RESULTS = None


def kernel(**inputs):
    global LAST_RESULTS
    if "nc" not in _CACHE:
        _CACHE["nc"] = build()
    nc = _CACHE["nc"]
    in_maps = host_prep(inputs)
    trace = bool(int(os.environ.get("KERNEL_TRACE", "0")))
    res = run_bass_kernel_spmd(nc, in_maps, list(range(NCORE)), trace=trace)
    LAST_RESULTS = res
    outs = []
    for c in range(NCORE):
        lt = res.results[c]["logitsT"]           # [V, NT] fp32
        outs.append(np.ascontiguousarray(lt.T).reshape(BL, T, V))
    return np.concatenate(outs, axis=0).astype(np.float32)


# revision 15
# speedup vs baseline: 1.3408x; 1.3408x over previous
"""Trainium2 Bass kernel for nn_BigramModel (6-layer GPT-style transformer).

Strategy: data-parallel over 8 NeuronCores (B=128 -> 16 batch rows/core).
On-device layout is feature-major ("transposed"): activations live as
[C, tokens] so every linear layer is a plain PE matmul chain with no
transposes anywhere.  All matmuls run in fp16 (fp32 PSUM accumulation);
the residual stream, layernorm statistics and softmax denominators stay
fp32.

 - LayerNorm: per-token stats via ones-column PE matmuls (sum(x), sum(x^2)),
   rsqrt computed as exp(-0.5*ln(q)) so ACT only ever needs the
   natural_log_exp table set; normalization applied as x*Abc + Bbc where
   Abc/Bbc are rank-1 (gamma x A-row) built by K=1 PE matmuls.
 - Attention: scores computed transposed (scoresT[k, q]) per (batch, head),
   causal masking on the fp16 exp tiles, denominators broadcast with
   ones-block matmuls, 1/denom folded into the AV output eviction.
"""

import os
import numpy as np
from contextlib import ExitStack

import concourse.bass as bass
import concourse.tile as tile
from concourse import bacc, mybir
from concourse.bass_utils import run_bass_kernel_spmd

AF = mybir.ActivationFunctionType
ALU = mybir.AluOpType
f16 = mybir.dt.float16
f32 = mybir.dt.float32

B, T, V, C, H, HD, L = 128, 256, 100, 384, 6, 64, 6
FF = 4 * C
NCORE = 8
BL = B // NCORE          # 16 batch rows per core
NT = BL * T              # 4096 tokens per core
CH = 512                 # tokens per pipeline chunk (2 batch rows)
NCH = NT // CH           # 8 chunks
CC = C // 128            # 3 feature chunks
FC = FF // 128           # 12 ffn feature chunks
LNEPS_C = (384.0 ** 2) * 1e-5   # eps term for 384^2 * var
LOG384 = float(np.log(384.0))


def _patch_act_tables():
    """Force every ACT instruction onto the natural_log_exp_and_others table
    set (covers exp/ln/copy/identity) so the kernel loads exactly one ACT
    table instead of thrashing between exp- and ln-sets at every layernorm."""
    import concourse.bacc as _bacc
    if getattr(_bacc, "_act_tables_patched", False):
        return
    orig = _bacc.get_activation_tables

    def patched(arch):
        tabs = orig(arch)
        keep = "natural_log_exp_and_others"
        if keep in tabs:
            tabs = {k: (v if k == keep else set()) for k, v in tabs.items()}
        return tabs

    _bacc.get_activation_tables = patched
    _bacc._act_tables_patched = True


def build(n_layers=L, n_chunks=NCH, taps=()):
    _patch_act_tables()
    nc = bacc.Bacc("TRN2", target_bir_lowering=False, debug=False)

    x0_d = nc.dram_tensor("x0T", [C, NT], f32, kind="ExternalInput")
    wq_d, wk_d, wv_d, wo_d, w1_d, w2_d = [], [], [], [], [], []
    bias4_d, vbr_d, b1e_d = [], [], []
    for l in range(n_layers):
        wq_d.append(nc.dram_tensor(f"wq{l}", [C, C], f16, kind="ExternalInput"))
        wk_d.append(nc.dram_tensor(f"wk{l}", [C, C], f16, kind="ExternalInput"))
        wv_d.append(nc.dram_tensor(f"wv{l}", [C, C], f16, kind="ExternalInput"))
        wo_d.append(nc.dram_tensor(f"wo{l}", [C, C], f16, kind="ExternalInput"))
        w1_d.append(nc.dram_tensor(f"w1{l}", [C, FF], f16, kind="ExternalInput"))
        w2_d.append(nc.dram_tensor(f"w2{l}", [FF, C], f16, kind="ExternalInput"))
        bias4_d.append(nc.dram_tensor(f"bias4_{l}", [C, 6], f32, kind="ExternalInput"))
        vbr_d.append(nc.dram_tensor(f"vbr{l}", [1, C], f16, kind="ExternalInput"))
        b1e_d.append(nc.dram_tensor(f"b1e{l}", [128, FC], f32, kind="ExternalInput"))
    mska_d = nc.dram_tensor("mska", [128, 768], f16, kind="ExternalInput")
    gfc_d = nc.dram_tensor("gfc", [C, 1], f32, kind="ExternalInput")
    wlm_d = nc.dram_tensor("wlm", [C, V], f16, kind="ExternalInput")
    lmb_d = nc.dram_tensor("lmb", [V, 1], f32, kind="ExternalInput")
    out_d = nc.dram_tensor("logitsT", [V, NT], f32, kind="ExternalOutput")
    tap_d = {}
    for tname in taps:
        tap_d[tname] = nc.dram_tensor(tname, [C, NT], f32, kind="ExternalOutput")

    with tile.TileContext(nc) as tc, ExitStack() as ctx:
        const = ctx.enter_context(tc.tile_pool(name="const", bufs=1))
        wpool = ctx.enter_context(tc.tile_pool(name="w", bufs=2))
        xpool = ctx.enter_context(tc.tile_pool(name="x", bufs=1))
        sc = ctx.enter_context(tc.tile_pool(name="sc", bufs=2))
        lnp = ctx.enter_context(tc.tile_pool(name="lnp", bufs=3))
        xsp = ctx.enter_context(tc.tile_pool(name="xsp", bufs=3))
        vpool = ctx.enter_context(tc.tile_pool(name="vp", bufs=5))
        mpool = ctx.enter_context(tc.tile_pool(name="m1p", bufs=13))
        epool = ctx.enter_context(tc.tile_pool(name="ep", bufs=5))
        psA = ctx.enter_context(tc.tile_pool(name="psA", bufs=3, space="PSUM"))
        psQ = ctx.enter_context(tc.tile_pool(name="psQ", bufs=1, space="PSUM"))
        psB = ctx.enter_context(tc.tile_pool(name="psB", bufs=2, space="PSUM"))
        psS = ctx.enter_context(tc.tile_pool(name="psS", bufs=2, space="PSUM"))

        # ---- constants
        mska = const.tile([128, 768], f16)           # causal masks for e-mega
        nc.sync.dma_start(mska[:], mska_d.ap())
        onesL = const.tile([128, 128], f16)          # cols 0-63 ones
        nc.vector.memset(onesL[:], 0.0)
        nc.vector.memset(onesL[:, 0:64], 1.0)
        onesR = const.tile([128, 128], f16)          # cols 64-127 ones
        nc.vector.memset(onesR[:], 0.0)
        nc.vector.memset(onesR[:, 64:128], 1.0)
        ones_col = const.tile([128, 1], f16)
        nc.vector.memset(ones_col[:], 1.0)
        ones_row = const.tile([1, 128], f16)
        nc.vector.memset(ones_row[:], 1.0)
        lneps_t = const.tile([128, 1], f32)
        nc.vector.memset(lneps_t[:], LNEPS_C)
        log384_t = const.tile([128, 1], f32)
        nc.vector.memset(log384_t[:], LOG384)
        gfc = []
        for i in range(CC):
            g = const.tile([128, 1], f32, tag=f"gfc{i}")
            nc.sync.dma_start(g[:], gfc_d.ap()[128 * i:128 * (i + 1), :])
            gfc.append(g)
        wlm_t = []
        for i in range(CC):
            w = const.tile([128, V], f16, tag=f"wlm{i}")
            nc.sync.dma_start(w[:], wlm_d.ap()[128 * i:128 * (i + 1), :])
            wlm_t.append(w)
        lmb_t = const.tile([128, 1], f32)
        nc.sync.dma_start(lmb_t[0:V, :], lmb_d.ap())

        # ---- resident residual stream (feature-major, fp32)
        x_t = []
        for i in range(CC):
            xt = xpool.tile([128, NT], f32, tag=f"x{i}")
            nc.sync.dma_start(xt[:], x0_d.ap()[128 * i:128 * (i + 1), :])
            x_t.append(xt)

        def layer_norm_chunk(t0, g_aps):
            """LN of x[:, t0:t0+CH]; returns 3 fp16 [128, CH] tiles (gamma applied,
            beta NOT applied - folded into consumers). g_aps: per-chunk [128,1] f32."""
            stats = psS.tile([128, CH], f32, tag="st")
            for i in range(CC):
                xs = x_t[i][:, t0:t0 + CH]
                x16 = xsp.tile([128, CH], f16, tag=f"x16_{i}")
                nc.scalar.copy(x16[:], xs)
                xsq = sc.tile([128, CH], f16, tag=f"xsq_{i}")
                nc.vector.tensor_mul(xsq[:], xs, xs)
                nc.tensor.matmul(stats[0:1, :], ones_col[:], x16[:],
                                 start=(i == 0), stop=(i == CC - 1))
                nc.tensor.matmul(stats[32:33, :], ones_col[:], xsq[:],
                                 start=(i == 0), stop=(i == CC - 1))
            sq = lnp.tile([1, CH], f32, tag="sq")
            nc.scalar.activation(sq[:], stats[0:1, :], AF.Square)
            qr = lnp.tile([1, CH], f32, tag="qr")
            nc.vector.scalar_tensor_tensor(qr[:], stats[32:33, :], 384.0, sq[:],
                                           op0=ALU.mult, op1=ALU.subtract)
            lg = lnp.tile([1, CH], f32, tag="lgr")
            nc.scalar.activation(lg[:], qr[:], AF.Ln, bias=lneps_t[0:1, :], scale=1.0)
            ar = lnp.tile([1, CH], f16, tag="ar")
            nc.scalar.activation(ar[:], lg[:], AF.Exp, bias=log384_t[0:1, :], scale=-0.5)
            br = lnp.tile([1, CH], f16, tag="br")
            nc.vector.scalar_tensor_tensor(br[:], stats[0:1, :], -1.0 / 384.0, ar[:],
                                           op0=ALU.mult, op1=ALU.mult)
            ab = psB.tile([128, CH], f32, tag="b")
            nc.tensor.matmul(ab[:], ones_row[:], ar[:], start=True, stop=True)
            bb = psB.tile([128, CH], f32, tag="b")
            nc.tensor.matmul(bb[:], ones_row[:], br[:], start=True, stop=True)
            xh = []
            for i in range(CC):
                tt = sc.tile([128, CH], f32, tag="t1")
                nc.vector.scalar_tensor_tensor(tt[:], x_t[i][:, t0:t0 + CH],
                                               g_aps[i], ab[:],
                                               op0=ALU.mult, op1=ALU.mult)
                xh_i = sc.tile([128, CH], f16, tag=f"xh{i}")
                nc.vector.scalar_tensor_tensor(xh_i[:], bb[:], g_aps[i], tt[:],
                                               op0=ALU.mult, op1=ALU.add)
                xh.append(xh_i)
            return xh

        for l in range(n_layers):
            # ---- load layer weights (double-buffered via tags)
            def wload(dram, n_t, width, name):
                ts = []
                for i in range(n_t):
                    w = wpool.tile([128, width], f16, tag=f"{name}{i}")
                    nc.sync.dma_start(w[:], dram.ap()[128 * i:128 * (i + 1), :])
                    ts.append(w)
                return ts
            wq_t = wload(wq_d[l], CC, C, "wq")
            wk_t = wload(wk_d[l], CC, C, "wk")
            wv_t = wload(wv_d[l], CC, C, "wv")
            wo_t = wload(wo_d[l], CC, C, "wo")
            w1_t = wload(w1_d[l], CC, FF, "w1")
            w2_t = wload(w2_d[l], FC, C, "w2")
            bias4_t = []
            for i in range(CC):
                bt = wpool.tile([128, 6], f32, tag=f"b4_{i}")
                nc.sync.dma_start(bt[:], bias4_d[l].ap()[128 * i:128 * (i + 1), :])
                bias4_t.append(bt)
            vbr = wpool.tile([1, C], f16, tag="vbr")
            nc.sync.dma_start(vbr[:], vbr_d[l].ap())
            b1e = wpool.tile([128, FC], f32, tag="b1e")
            nc.sync.dma_start(b1e[:], b1e_d[l].ap())

            def qkv_chunk(ch):
                t0 = ch * CH
                xh = layer_norm_chunk(t0, [bias4_t[i][:, 4:5] for i in range(CC)])
                qt, kt = [], []
                for j in range(CC):           # output head-pair tiles
                    qp = psQ.tile([128, CH], f32, tag="q")
                    for i in range(CC):
                        nc.tensor.matmul(qp[:], wq_t[i][:, 128 * j:128 * (j + 1)],
                                         xh[i][:], start=(i == 0), stop=(i == CC - 1))
                    q16 = sc.tile([128, CH], f16, tag=f"qt{j}")
                    nc.scalar.activation(q16[:], qp[:], AF.Identity,
                                         bias=bias4_t[j][:, 0:1], scale=1.0)
                    qt.append(q16)
                    kp = psQ.tile([128, CH], f32, tag="q")
                    for i in range(CC):
                        nc.tensor.matmul(kp[:], wk_t[i][:, 128 * j:128 * (j + 1)],
                                         xh[i][:], start=(i == 0), stop=(i == CC - 1))
                    k16 = sc.tile([128, CH], f16, tag=f"kt{j}")
                    nc.scalar.activation(k16[:], kp[:], AF.Identity,
                                         bias=bias4_t[j][:, 1:2], scale=1.0)
                    kt.append(k16)
                vt = []
                for tb in range(CH // 128):   # token-major v tiles [128 tok, C]
                    vp = psB.tile([128, C], f32, tag="b")
                    for i in range(CC):
                        nc.tensor.matmul(vp[:], xh[i][:, 128 * tb:128 * (tb + 1)],
                                         wv_t[i][:], start=(i == 0), stop=False)
                    nc.tensor.matmul(vp[:], ones_row[:], vbr[:], start=False, stop=True)
                    v16 = vpool.tile([128, C], f16, tag="v16")
                    nc.scalar.copy(v16[:], vp[:])
                    vt.append(v16)
                return qt, kt, vt

            pend = qkv_chunk(0)
            for ch in range(n_chunks):
                t0 = ch * CH
                qt, kt, vt = pend

                # ======== attention ========
                oT = []
                for j in range(CC):
                    ot = sc.tile([128, CH], f16, tag=f"oT{j}")
                    oT.append(ot)
                for hb in range(CH // T):     # batch row within chunk
                    q0 = hb * T
                    for hp in range(CC):      # head pair
                        em = epool.tile([128, 768], f16, tag="e")
                        for hh in range(2):
                            rb = 64 * hh
                            sp = psA.tile([128, 2 * T], f32, tag="a")
                            nc.tensor.matmul(
                                sp[:, 0:T],
                                kt[hp][rb:rb + 64, q0:q0 + 128],
                                qt[hp][rb:rb + 64, q0:q0 + T],
                                start=True, stop=True)
                            nc.tensor.matmul(
                                sp[:, T:T + 128],
                                kt[hp][rb:rb + 64, q0 + 128:q0 + T],
                                qt[hp][rb:rb + 64, q0 + 128:q0 + T],
                                start=True, stop=True)
                            nc.scalar.activation(em[:, 384 * hh:384 * (hh + 1)],
                                                 sp[:, 0:384], AF.Exp, scale=0.125)
                        nc.vector.tensor_mul(em[:], em[:], mska[:])
                        ee = lambda hh, ktb: (
                            em[:, 384 * hh:384 * hh + T] if ktb == 0
                            else em[:, 384 * hh + T:384 * hh + T + 128])
                        db = psA.tile([128, 2 * T], f32, tag="a")
                        nc.tensor.matmul(db[:, 0:T], onesL[:], ee(0, 0), start=True, stop=False)
                        nc.tensor.matmul(db[:, 0:T], onesR[:], ee(1, 0), start=False, stop=False)
                        nc.tensor.matmul(db[:, 128:T], onesL[:], ee(0, 1), start=False, stop=False)
                        nc.tensor.matmul(db[:, 128:T], onesR[:], ee(1, 1), start=False, stop=True)
                        rd = sc.tile([128, T], f32, tag="rd")
                        nc.vector.reciprocal_approx_fast(rd[:], db[:, 0:T])
                        av = psA.tile([128, 2 * T], f32, tag="a")
                        for hh in range(2):
                            rb = 64 * hh
                            nc.tensor.matmul(
                                av[rb:rb + 64, 0:T],
                                vt[hb * 2][:, 128 * hp + rb:128 * hp + rb + 64],
                                ee(hh, 0), start=True, stop=False)
                            nc.tensor.matmul(
                                av[rb:rb + 64, 128:T],
                                vt[hb * 2 + 1][:, 128 * hp + rb:128 * hp + rb + 64],
                                ee(hh, 1), start=False, stop=True)
                        nc.vector.tensor_mul(oT[hp][:, q0:q0 + T], av[:, 0:T], rd[:])

                # ======== Wo + residual ========
                for co in range(CC):
                    dl = psB.tile([128, CH], f32, tag="b")
                    for ci in range(CC):
                        nc.tensor.matmul(dl[:], wo_t[ci][:, 128 * co:128 * (co + 1)],
                                         oT[ci][:], start=(ci == 0), stop=(ci == CC - 1))
                    nc.vector.scalar_tensor_tensor(
                        x_t[co][:, t0:t0 + CH], dl[:], bias4_t[co][:, 2:3],
                        x_t[co][:, t0:t0 + CH], op0=ALU.add, op1=ALU.add)

                # ======== next-chunk QKV (interleaved for PE overlap) ========
                if ch + 1 < n_chunks:
                    pend = qkv_chunk(ch + 1)

                # ======== LN2 + FFN ========
                xh2 = layer_norm_chunk(t0, [bias4_t[i][:, 5:6] for i in range(CC)])
                m1 = []
                for fo in range(FC):
                    mp = psB.tile([128, CH], f32, tag="b")
                    for ci in range(CC):
                        nc.tensor.matmul(mp[:], w1_t[ci][:, 128 * fo:128 * (fo + 1)],
                                         xh2[ci][:], start=(ci == 0), stop=(ci == CC - 1))
                    m16 = mpool.tile([128, CH], f16, tag="m1")
                    nc.scalar.activation(m16[:], mp[:], AF.Relu,
                                         bias=b1e[:, fo:fo + 1], scale=1.0)
                    m1.append(m16)
                for co in range(CC):
                    dl = psB.tile([128, CH], f32, tag="b")
                    for fo in range(FC):
                        nc.tensor.matmul(dl[:], w2_t[fo][:, 128 * co:128 * (co + 1)],
                                         m1[fo][:], start=(fo == 0), stop=(fo == FC - 1))
                    nc.vector.scalar_tensor_tensor(
                        x_t[co][:, t0:t0 + CH], dl[:], bias4_t[co][:, 3:4],
                        x_t[co][:, t0:t0 + CH], op0=ALU.add, op1=ALU.add)

        # ======== final LN + LM head ========
        for ch in range(n_chunks):
            t0 = ch * CH
            xhf = layer_norm_chunk(t0, [g[:, 0:1] for g in gfc])
            lp = psB.tile([128, CH], f32, tag="b")
            for ci in range(CC):
                nc.tensor.matmul(lp[0:V, :], wlm_t[ci][:], xhf[ci][:],
                                 start=(ci == 0), stop=(ci == CC - 1))
            lo = sc.tile([128, CH], f32, tag="lo")
            nc.scalar.activation(lo[0:V, :], lp[0:V, :], AF.Identity,
                                 bias=lmb_t[0:V, :], scale=1.0)
            nc.sync.dma_start(out_d.ap()[:, t0:t0 + CH], lo[0:V, :])

        for tname in taps:
            for i in range(CC):
                nc.sync.dma_start(tap_d[tname].ap()[128 * i:128 * (i + 1), :], x_t[i][:])

    nc.compile()
    return nc


def host_prep(inputs, n_layers=L):
    """Build the per-core input maps from the full model inputs."""
    idx = np.asarray(inputs["idx"])
    embed = np.asarray(inputs["embed"], dtype=np.float32)
    pos = np.asarray(inputs["pos_embed"], dtype=np.float32)
    Wq = np.asarray(inputs["Wq"], dtype=np.float32)
    Wk = np.asarray(inputs["Wk"], dtype=np.float32)
    Wv = np.asarray(inputs["Wv"], dtype=np.float32)
    Wo = np.asarray(inputs["Wo"], dtype=np.float32)
    bo = np.asarray(inputs["bo"], dtype=np.float32)
    W1 = np.asarray(inputs["W1"], dtype=np.float32)
    b1 = np.asarray(inputs["b1"], dtype=np.float32)
    W2 = np.asarray(inputs["W2"], dtype=np.float32)
    b2 = np.asarray(inputs["b2"], dtype=np.float32)
    ln1_g = np.asarray(inputs["ln1_g"], dtype=np.float32)
    ln1_b = np.asarray(inputs["ln1_b"], dtype=np.float32)
    ln2_g = np.asarray(inputs["ln2_g"], dtype=np.float32)
    ln2_b = np.asarray(inputs["ln2_b"], dtype=np.float32)
    lnf_g = np.asarray(inputs["lnf_g"], dtype=np.float32)
    lnf_b = np.asarray(inputs["lnf_b"], dtype=np.float32)
    Wlm = np.asarray(inputs["Wlm"], dtype=np.float32)
    blm = np.asarray(inputs["blm"], dtype=np.float32)

    x0 = embed[idx] + pos[None, :T, :]          # [B, T, C] fp32

    shared = {}
    for l in range(n_layers):
        shared[f"wq{l}"] = Wq[l].astype(np.float16)
        shared[f"wk{l}"] = Wk[l].astype(np.float16)
        shared[f"wv{l}"] = Wv[l].astype(np.float16)
        shared[f"wo{l}"] = Wo[l].astype(np.float16)
        shared[f"w1{l}"] = W1[l].astype(np.float16)
        shared[f"w2{l}"] = W2[l].astype(np.float16)
        shared[f"bias4_{l}"] = np.stack(
            [Wq[l].T @ ln1_b[l], Wk[l].T @ ln1_b[l], bo[l], b2[l],
             ln1_g[l], ln2_g[l]],
            axis=1).astype(np.float32)
        shared[f"vbr{l}"] = (ln1_b[l] @ Wv[l])[None, :].astype(np.float16)
        b1e = b1[l] + W1[l].T @ ln2_b[l]
        shared[f"b1e{l}"] = np.ascontiguousarray(
            b1e.reshape(FC, 128).T).astype(np.float32)
    k_i = np.arange(128)[:, None]
    q_i = np.arange(128)[None, :]
    tri = (k_i <= q_i).astype(np.float16)
    ones = np.ones((128, 128), np.float16)
    zeros = np.zeros((128, 128), np.float16)
    half = np.concatenate([tri, ones, tri], axis=1)
    shared["mska"] = np.concatenate([half, half], axis=1)
    shared["gfc"] = lnf_g[:, None].astype(np.float32)
    shared["wlm"] = Wlm.astype(np.float16)
    shared["lmb"] = (blm + Wlm.T @ lnf_b)[:, None].astype(np.float32)

    in_maps = []
    for c in range(NCORE):
        x0c = x0[c * BL:(c + 1) * BL].reshape(NT, C).T
        m = dict(shared)
        m["x0T"] = np.ascontiguousarray(x0c, dtype=np.float32)
        in_maps.append(m)
    return in_maps


_CACHE = {}
LAST_RESULTS = None


def kernel(**inputs):
    global LAST_RESULTS
    if "nc" not in _CACHE:
        _CACHE["nc"] = build()
    nc = _CACHE["nc"]
    in_maps = host_prep(inputs)
    trace = bool(int(os.environ.get("KERNEL_TRACE", "0")))
    res = run_bass_kernel_spmd(nc, in_maps, list(range(NCORE)), trace=trace)
    LAST_RESULTS = res
    outs = []
    for c in range(NCORE):
        lt = res.results[c]["logitsT"]           # [V, NT] fp32
        outs.append(np.ascontiguousarray(lt.T).reshape(BL, T, V))
    return np.concatenate(outs, axis=0).astype(np.float32)
